# revision 3
# baseline (speedup 1.0000x reference)
"""3D Haar DWT low-pass (DWT3DTiny) Trainium2 kernel.

The reference applies the Haar rec_lo filter [s, s] (s = sqrt(2)/2) with
stride-2 downsampling along t, h, w for every channel.  That is exactly a
2x2x2 box sum scaled by s^3 = 2**-1.5:

    out[ts, hs, ws, c] = 2**-1.5 * sum_{dt,dh,dw in {0,1}} x[2ts+dt, 2hs+dh, 2ws+dw, c]

Sharding: along t (pure data-parallel, t-pairs never cross a core
boundary since 32 / 8 = 4 rows per core), contiguous host-side slices.

v2 layout (from NTFF packet analysis of the v1 kernel, 97.1 us):
the 16 SDMA engines are the wall: they were >97% occupied in-span and
per-packet throughput is size-dependent (8 KiB -> 26.0, asymptote
27.35 GB/s/engine).  So v2 maximizes descriptor size everywhere:
  * bulk chunks are (t-row, 256-h-row-block) = 4 MiB tiles whose
    partition p holds h rows (2p, 2p+1) full-width -> two 16 KiB
    contiguous descriptors per partition;
  * h/t adds run in place inside the input tiles (SBUF fits bufs=2 of
    the 4 MiB pair), w-add into a small tile, ACT scales + stores with
    8 KiB descriptors;
  * only the final chunk's second t-row is split into graduated
    w-pieces [256,128,64,48,16] so the post-last-load drain is short
    while keeping most tail bytes on >=4 KiB descriptors;
  * loads on the SP HWDGE ring, stores on the ACT ring (sharing one
    ring head-of-line blocks loads behind stores);
  * dead const-tile memsets stripped from the init preamble (~9 us of
    GpSimd startup the all-engine barrier otherwise waits on).
"""

import numpy as np

import concourse.bacc as bacc
import concourse.mybir as mybir
from concourse.bass_utils import run_bass_kernel_spmd
from concourse.tile import TileContext

N_CORES = 8
T, H, W, C = 32, 512, 512, 8
TS = T // N_CORES  # t rows per core
SCALE = float(2.0 ** -1.5)
TAIL_WI = [64, 64, 64, 64, 64, 64, 64, 32, 32]  # final chunk's b pieces

_CACHE: dict = {}


def _build_nc() -> bacc.Bacc:
    nc = bacc.Bacc("TRN2", target_bir_lowering=False)
    x = nc.dram_tensor("x", [TS, H, W, C], mybir.dt.float32, kind="ExternalInput")
    y = nc.dram_tensor(
        "y", [TS // 2, H // 2, W // 2, C], mybir.dt.float32, kind="ExternalOutput"
    )

    # h = gb*256 + p*2 + two; per (t, gb, p): rows 2p, 2p+1 full-width are
    # two adjacent 16 KiB contiguous descriptors.
    xq = x.rearrange("t (gb p two) w c -> t gb p two (w c)", p=128, two=2)
    # output row g = gb*128 + p: 256 v * 8 c = 8 KiB contiguous per partition
    yq = y.rearrange("s (gb p) w c -> s gb p (w c)", p=128)

    WC = W * C  # 4096 f32 per h row

    chunks = [(tp, gb) for tp in range(TS // 2) for gb in range(H // 256)]

    with TileContext(nc) as tc:
        with (
            tc.tile_pool(name="pin", bufs=2) as pin,
            tc.tile_pool(name="pw", bufs=3) as pw,
            tc.tile_pool(name="ptail", bufs=1) as pt,
        ):

            def wadd_scale_store(src, hw, wtile, ydst):
                # w-pair add (wi = v*2 + dw), then ACT scale + store
                hv = src.rearrange("p (v two c) -> p v two c", two=2, c=C)
                wv = wtile.rearrange("p (v c) -> p v c", c=C)
                nc.vector.tensor_add(out=wv[:], in0=hv[:, :, 0], in1=hv[:, :, 1])
                nc.scalar.mul(wtile[:], wtile[:], SCALE)
                nc.scalar.dma_start(out=ydst, in_=wtile[:])

            for ci, (tp, gb) in enumerate(chunks):
                if ci < len(chunks) - 1:
                    a = pin.tile([128, 2, WC], mybir.dt.float32, tag="a")
                    b = pin.tile([128, 2, WC], mybir.dt.float32, tag="b")
                    nc.sync.dma_start(out=a[:], in_=xq[2 * tp, gb])
                    nc.sync.dma_start(out=b[:], in_=xq[2 * tp + 1, gb])
                    # h-pair adds, in place (out index trails reads)
                    nc.vector.tensor_add(out=a[:, 0], in0=a[:, 0], in1=a[:, 1])
                    nc.vector.tensor_add(out=b[:, 0], in0=b[:, 0], in1=b[:, 1])
                    # t-pair add
                    nc.vector.tensor_add(out=a[:, 0], in0=a[:, 0], in1=b[:, 0])
                    ws = pw.tile([128, WC // 2], mybir.dt.float32, tag="w")
                    wadd_scale_store(a[:, 0], WC, ws, yq[tp, gb])
                else:
                    # final chunk: full a (t row 2tp), graduated b pieces
                    a = pin.tile([128, 2, WC], mybir.dt.float32, tag="a")
                    nc.sync.dma_start(out=a[:], in_=xq[2 * tp, gb])
                    nc.vector.tensor_add(out=a[:, 0], in0=a[:, 0], in1=a[:, 1])
                    w0 = 0
                    for k, wi in enumerate(TAIL_WI):
                        wc = wi * C
                        bp = pt.tile([128, 2, wc], mybir.dt.float32, tag=f"tb{k}")
                        nc.sync.dma_start(
                            out=bp[:],
                            in_=xq[2 * tp + 1, gb, :, :, w0 * C : (w0 + wi) * C],
                        )
                        nc.vector.tensor_add(out=bp[:, 0], in0=bp[:, 0], in1=bp[:, 1])
                        nc.vector.tensor_add(
                            out=bp[:, 0],
                            in0=bp[:, 0],
                            in1=a[:, 0, w0 * C : (w0 + wi) * C],
                        )
                        wt = pt.tile([128, wc // 2], mybir.dt.float32, tag=f"tw{k}")
                        wadd_scale_store(
                            bp[:, 0],
                            wc,
                            wt,
                            yq[tp, gb, :, (w0 // 2) * C : ((w0 + wi) // 2) * C],
                        )
                        w0 += wi

    _strip_init_preamble(nc)
    if not nc.is_finalized():
        nc.finalize()  # Bacc.compile: event-sem split (1 wait/inst), reg alloc
    return nc


def _strip_init_preamble(nc) -> None:
    """Drop the four Bass.__init__ const-tile memsets from block 0.  Nothing
    in this kernel reads the const tiles (scalar.mul uses an immediate), yet
    the initial all-engine barrier waits on the GpSimd engine executing them,
    which costs ~9 us of Q7 startup on HW.  The drains and the all-engine
    barrier themselves are kept intact."""
    b0 = nc.main_func.blocks[0]
    b0.instructions[:] = [
        ins for ins in b0.instructions if type(ins).__name__ != "InstMemset"
    ]


def kernel(x) -> np.ndarray:
    x = np.asarray(x, dtype=np.float32)
    assert x.shape == (T, H, W, C), x.shape

    if "nc" not in _CACHE:
        _CACHE["nc"] = _build_nc()
    nc = _CACHE["nc"]

    in_maps = [
        {"x": np.ascontiguousarray(x[i * TS : (i + 1) * TS])} for i in range(N_CORES)
    ]
    res = run_bass_kernel_spmd(nc, in_maps, core_ids=list(range(N_CORES)))
    return np.concatenate([r["y"] for r in res.results], axis=0)
